# revision 1
# baseline (speedup 1.0000x reference)
"""CapsuleLayer dynamic-routing kernel for Trainium2 (8 NeuronCores).

Problem: inputs [B=32, I=2048, J=16], W [N=64, I=2048, D=32, J=16], routings=3.
  inputs_hat[b,n,i,d] = sum_j inputs[b,i,j] * W[n,i,d,j]
  3 rounds of routing (softmax over n, weighted sum over i, squash over d).

Strategy: shard the input-capsule axis I across the 8 cores (I_loc=256).
Each core recomputes its ihat shard from W each round (W streamed from HBM
as bf16 hi/lo pairs; ihat never hits DRAM), keeps its b-state [*, n, i_loc]
in SBUF, and the only cross-core data is the [B, N, D] partial sum s,
AllReduced (256 KB) once per round. Output replicated; host takes core 0's.

Matmuls run in bf16 with error compensation: x = xh + xl, W = Wh + Wl
(each bf16); rounds 1-2 accumulate xh*Wh + xh*Wl + xl*Wh in fp32 PSUM
(error ~2^-18). Round 0 uses xh*Wh only (it feeds logits, not the output).

On-chip layout per round, per group of 4 input capsules i:
  PE: col-tiled matmuls (tile_position=(0,32c)), K=j=16, M=b=32,
      Nf=(n,d)=2048 -> PSUM H-tile [128=(4i,32b), 2048=(64n,32d)]
  DVE/ACT: y = sum_d H*v ; b += y ; c = softmax_n(b) ; tmp2 = c*H
  PE: s_psum += selector.T @ tmp2  (folds partition groups AND sums over i)
"""

import sys

for p in ("/opt/trn_rl_repo",):
    if p not in sys.path:
        sys.path.insert(0, p)

import ml_dtypes
import numpy as np

import concourse.bacc as bacc
import concourse.mybir as mybir
import concourse.tile as tile
from concourse.bass_utils import run_bass_kernel_spmd

# problem constants (hardcoded per harness contract)
B, N, I, D, J = 32, 64, 2048, 32, 16
R = 3  # routings
CORES = 8
I_LOC = I // CORES  # 256
ND = N * D  # 2048
EPS = 1e-7

F32 = mybir.dt.float32
BF16 = mybir.dt.bfloat16
FX = mybir.AxisListType.X
ADD = mybir.AluOpType.add
MAX = mybir.AluOpType.max
ACT = mybir.ActivationFunctionType

GROUPS = I_LOC // 4  # 64 groups of 4 capsules per round
NQ = ND // 512  # free-dim quarters per capsule


def _squash_build(nc, vbpool, smalls, kp, s4, eps_ap):
    """s4: [128, 2048] tile holding s (replicated x4 on partition groups).
    Returns vb [128, 2048] = squash(s) broadcast tile (same replication)."""
    sq = smalls.tile([128, N], F32, tag="sq_sq")
    for h in range(2):
        s2 = kp.tile([128, ND // 2], F32, tag="tmp")
        nc.scalar.square(s2[:], s4[:, h * (ND // 2):(h + 1) * (ND // 2)])
        nc.vector.tensor_reduce(
            sq[:, 32 * h:32 * (h + 1)],
            s2[:].rearrange("p (n d) -> p n d", d=D), axis=FX, op=ADD)
    # t = sqrt(sq + eps)
    t = smalls.tile([128, N], F32, tag="sq_t")
    nc.scalar.activation(t[:], sq[:], ACT.Sqrt, bias=eps_ap)
    # q1 = 1 + sq
    q1 = smalls.tile([128, N], F32, tag="sq_q1")
    nc.scalar.activation(q1[:], sq[:], ACT.Identity, bias=1.0)
    den = smalls.tile([128, N], F32, tag="sq_den")
    nc.vector.tensor_mul(den[:], q1[:], t[:])
    rs = smalls.tile([128, N], F32, tag="sq_rs")
    nc.vector.reciprocal(rs[:], den[:])
    scale = smalls.tile([128, N], F32, tag="sq_scale")
    nc.vector.tensor_mul(scale[:], sq[:], rs[:])
    vb = vbpool.tile([128, ND], F32, tag="sq_vb")
    nc.vector.tensor_mul(
        vb[:].rearrange("p (n d) -> p n d", d=D),
        s4[:].rearrange("p (n d) -> p n d", d=D),
        scale[:, :, None].broadcast_to([128, N, D]),
    )
    return vb


def build_kernel():
    nc = bacc.Bacc("TRN2", target_bir_lowering=False, debug=False)

    xth = nc.dram_tensor("xth", [I_LOC * J, B], BF16, kind="ExternalInput")
    xtl = nc.dram_tensor("xtl", [I_LOC * J, B], BF16, kind="ExternalInput")
    wth = nc.dram_tensor("wth", [I_LOC * J, ND], BF16, kind="ExternalInput")
    wtl = nc.dram_tensor("wtl", [I_LOC * J, ND], BF16, kind="ExternalInput")
    out = nc.dram_tensor("out", [B, N, D], F32, kind="ExternalOutput")

    # collective bounce buffers (one pair per round)
    s_in = [nc.dram_tensor(f"s_in{r}", [B, ND], F32) for r in range(R)]
    s_out = [nc.dram_tensor(f"s_out{r}", [B, ND], F32, addr_space="Shared")
             for r in range(R)]

    wth_v = wth[:].rearrange("(i j) f -> j i f", j=J)
    wtl_v = wtl[:].rearrange("(i j) f -> j i f", j=J)

    with tile.TileContext(nc) as tc:
        with (
            tc.tile_pool(name="persist", bufs=1) as pp,
            tc.tile_pool(name="wsbp", bufs=3) as wsbp,
            tc.tile_pool(name="wgp", bufs=6) as wgp,
            tc.tile_pool(name="vbp", bufs=1) as vbp,
            tc.tile_pool(name="work", bufs=3) as kp,
            tc.tile_pool(name="t2p", bufs=6) as t2p,
            tc.tile_pool(name="hsbp", bufs=6) as hsbp,
            tc.tile_pool(name="s4p", bufs=1) as s4p,
            tc.tile_pool(name="pbig", bufs=1) as pbig,
            tc.tile_pool(name="small", bufs=3) as sp,
            tc.tile_pool(name="psum", bufs=2, space="PSUM") as psp,
            tc.tile_pool(name="psumB", bufs=1, space="PSUM") as psB,
        ):
            # ---- resident tiles ----
            # x chunks for round-0 fused einsum: [128=(8i,16j), 32 chunks, B]
            xsbh = pp.tile([128, I_LOC * J // 128, B], BF16, tag="xsbh")
            nc.sync.dma_start(
                xsbh[:], xth[:].rearrange("(k p) b -> p k b", p=128))
            xsbl = pp.tile([128, I_LOC * J // 128, B], BF16, tag="xsbl")
            nc.sync.dma_start(
                xsbl[:], xtl[:].rearrange("(k p) b -> p k b", p=128))
            # x for per-capsule matmuls: [16=j, I_LOC, B] (partitions 0-15)
            xah = pp.tile([16, I_LOC, B], BF16, tag="xah")
            nc.sync.dma_start(xah[:],
                              xth[:].rearrange("(i j) b -> j i b", j=J))
            xal = pp.tile([16, I_LOC, B], BF16, tag="xal")
            nc.sync.dma_start(xal[:],
                              xtl[:].rearrange("(i j) b -> j i b", j=J))

            # routing logits b: [128=(c,b), GROUPS, N]
            bstate = pp.tile([128, GROUPS, N], F32, tag="bstate")
            nc.gpsimd.memset(bstate[:], 0.0)
            eps_t = pp.tile([128, 1], F32, tag="eps")
            nc.gpsimd.memset(eps_t[:], EPS)
            # selector[p, m] = 1.0 if p % 32 == m  (partition-group fold)
            sel_i = pp.tile([128, B], mybir.dt.int32, tag="sel_i")
            nc.gpsimd.iota(sel_i[:], [[1, B]], channel_multiplier=-1)
            nc.vector.tensor_scalar(sel_i[:], sel_i[:], 31, None,
                                    op0=mybir.AluOpType.bitwise_and)
            sel = pp.tile([128, B], F32, tag="sel")
            nc.vector.tensor_scalar(sel[:], sel_i[:], 0, None,
                                    op0=mybir.AluOpType.is_equal)

            # ---------- round 0: c uniform -> s0 = (1/N) sum_i ihat ----------
            ps0 = psB.tile([B, ND], F32, tag="pss")
            n_chunks = I_LOC * J // 128  # 32
            for k in range(n_chunks):
                wsbh = wsbp.tile([128, ND], BF16, tag="wsb")
                nc.sync.dma_start(wsbh[:], wth[k * 128:(k + 1) * 128, :])
                wsbl = wsbp.tile([128, ND], BF16, tag="wsb")
                nc.sync.dma_start(wsbl[:], wtl[k * 128:(k + 1) * 128, :])
                prods0 = [(xsbh, wsbh, 0), (xsbh, wsbl, 1), (xsbl, wsbh, 2)]
                for xs_, ws_, pi in prods0:
                    for q in range(NQ):
                        nc.tensor.matmul(
                            ps0[:, q * 512:(q + 1) * 512],
                            xs_[:, k, :],
                            ws_[:, q * 512:(q + 1) * 512],
                            start=(k == 0 and pi == 0),
                            stop=(k == n_chunks - 1 and pi == 2),
                        )
            s_loc0 = pbig.tile([B, ND], F32, tag="s_loc")
            nc.scalar.mul(s_loc0[:], ps0[:], 1.0 / N)
            nc.sync.dma_start(s_in[0][:], s_loc0[:])
            nc.gpsimd.collective_compute(
                "AllReduce", ADD,
                replica_groups=[list(range(CORES))],
                ins=[s_in[0].ap().opt()], outs=[s_out[0].ap().opt()],
            )
            s4 = s4p.tile([128, ND], F32, tag="s4")
            for g4 in range(4):
                nc.sync.dma_start(s4[g4 * 32:(g4 + 1) * 32, :], s_out[0][:])
            vb = _squash_build(nc, vbp, sp, kp, s4, eps_t[:])

            # ---------- rounds 1, 2 ----------
            HF = ND // 2  # 1024: free-dim half (n 0-31 / n 32-63)
            for r in (1, 2):
                ps_s = psB.tile([B, ND], F32, tag="pss")
                pending = []  # previous group's tmp2 halves (fold delayed)

                def flush_fold(pend, last, _ps=ps_s):
                    g0, t2s = pend
                    for h in range(2):
                        for q in range(2):
                            f0 = h * HF + q * 512
                            nc.tensor.matmul(
                                _ps[:, f0:f0 + 512],
                                sel[:],
                                t2s[h][:, q * 512:(q + 1) * 512],
                                start=(g0 == 0),
                                stop=(last and h == 1 and q == 1),
                                skip_group_check=True,
                            )

                post = []  # groups whose softmax/tmp2 stage is deferred

                def stage_b(g, hsbs):
                    # softmax over n (|b| is O(1): no max-subtraction needed)
                    bsl = bstate[:, g, :]
                    e = sp.tile([128, N], F32, tag="e")
                    se = sp.tile([128, 1], F32, tag="se")
                    nc.scalar.activation(e[:], bsl, ACT.Exp,
                                         accum_out=se[:])
                    rcp = sp.tile([128, 1], F32, tag="rcp")
                    nc.vector.reciprocal(rcp[:], se[:])
                    cg = sp.tile([128, N], F32, tag="cg")
                    nc.vector.tensor_scalar_mul(cg[:], e[:], rcp[:])
                    # tmp2 = c * H  (folded into ps_s two iterations later)
                    pool_mul = (g % 6 != 5)
                    t2s = []
                    for h in range(2):
                        eng = nc.gpsimd if (h == 1 and pool_mul) else nc.vector
                        tmp2 = t2p.tile([128, HF], F32, tag="tmp2")
                        eng.tensor_mul(
                            tmp2[:].rearrange("p (n d) -> p n d", d=D),
                            hsbs[h][:].rearrange("p (n d) -> p n d", d=D),
                            cg[:, 32 * h:32 * (h + 1), None].broadcast_to(
                                [128, 32, D]),
                        )
                        t2s.append(tmp2)
                    pending.append((g, t2s))

                for g in range(GROUPS):
                    # W rows for capsules i = 4g..4g+3
                    wghs, wgls = [], []
                    for u in range(2):
                        wgh_ = wgp.tile([16, 2, ND], BF16, tag="wg")
                        nc.sync.dma_start(
                            wgh_[:], wth_v[:, 4 * g + 2 * u:4 * g + 2 * u + 2, :])
                        wghs.append(wgh_)
                        wgl_ = wgp.tile([16, 2, ND], BF16, tag="wg")
                        nc.sync.dma_start(
                            wgl_[:], wtl_v[:, 4 * g + 2 * u:4 * g + 2 * u + 2, :])
                        wgls.append(wgl_)
                    hsbs = []
                    pg0 = psp.tile([128, HF], F32, tag="pg")
                    pg1 = psp.tile([128, HF], F32, tag="pg")
                    pgs = [pg0, pg1]
                    for c in range(4):
                        i = 4 * g + c
                        wgh, wgl = wghs[c // 2], wgls[c // 2]
                        seq = [(xah, wgh, 0), (xah, wgl, 1), (xal, wgh, 2)]
                        for xa_, wg_, pi in seq:
                            for h in range(2):
                                for q in range(2):
                                    f0 = h * HF + q * 512
                                    nc.tensor.matmul(
                                        pgs[h][32 * c:32 * (c + 1),
                                               q * 512:(q + 1) * 512],
                                        xa_[:, i, :],
                                        wg_[:, c % 2, f0:f0 + 512],
                                        start=(pi == 0), stop=(pi == 2),
                                        tile_position=(0, 32 * c),
                                    )
                    for h in range(2):
                        # stage H half to SBUF on ScalarE; frees PSUM fast
                        hsb = hsbp.tile([128, HF], F32, tag="hsb")
                        nc.scalar.copy(hsb[:], pgs[h][:])
                        hsbs.append(hsb)
                    # fold tmp2 from two stage-B's back
                    if len(pending) >= 2:
                        flush_fold(pending.pop(0), False)
                    # y = sum_d H * v   (h1 muls on GpSimd most groups)
                    pool_mul = (g % 6 != 5)
                    y = sp.tile([128, N], F32, tag="y")
                    for h in range(2):
                        eng = nc.gpsimd if (h == 1 and pool_mul) else nc.vector
                        tmp = kp.tile([128, HF], F32, tag="tmp")
                        eng.tensor_mul(tmp[:], hsbs[h][:],
                                       vb[:, h * HF:(h + 1) * HF])
                        nc.vector.tensor_reduce(
                            y[:, 32 * h:32 * (h + 1)],
                            tmp[:].rearrange("p (n d) -> p n d", d=D),
                            axis=FX, op=ADD)
                    # b += y
                    bsl = bstate[:, g, :]
                    nc.vector.tensor_add(bsl, bsl, y[:])
                    # deferred softmax/tmp2 for the previous group
                    post.append((g, hsbs))
                    if len(post) >= 2:
                        stage_b(*post.pop(0))
                stage_b(*post.pop(0))
                flush_fold(pending.pop(0), False)
                flush_fold(pending.pop(0), False)
                flush_fold(pending.pop(0), True)

                s_loc = pbig.tile([B, ND], F32, tag="s_loc")
                nc.scalar.copy(s_loc[:], ps_s[:])
                nc.sync.dma_start(s_in[r][:], s_loc[:])
                nc.gpsimd.collective_compute(
                    "AllReduce", ADD,
                    replica_groups=[list(range(CORES))],
                    ins=[s_in[r].ap().opt()], outs=[s_out[r].ap().opt()],
                )
                s4 = s4p.tile([128, ND], F32, tag="s4")
                for g4 in range(4):
                    nc.sync.dma_start(s4[g4 * 32:(g4 + 1) * 32, :],
                                      s_out[r][:])
                vb = _squash_build(nc, vbp, sp, kp, s4, eps_t[:])

            # output = squash(s2) = vb rows 0..31
            nc.sync.dma_start(
                out[:].rearrange("b n d -> b (n d)"), vb[0:32, :])

    nc.compile()
    return nc


_NC_CACHE = {}


def _get_nc():
    if "nc" not in _NC_CACHE:
        _NC_CACHE["nc"] = build_kernel()
    return _NC_CACHE["nc"]


def _hi_lo(a):
    hi = a.astype(ml_dtypes.bfloat16)
    lo = (a - hi.astype(np.float32)).astype(ml_dtypes.bfloat16)
    return hi, lo


def _make_in_maps(inputs, W):
    inputs = np.ascontiguousarray(np.asarray(inputs, dtype=np.float32))
    W = np.ascontiguousarray(np.asarray(W, dtype=np.float32))
    assert inputs.shape == (B, I, J) and W.shape == (N, I, D, J)
    in_maps = []
    for c in range(CORES):
        sl = slice(c * I_LOC, (c + 1) * I_LOC)
        # xt: [(i j), b]
        x_t = np.ascontiguousarray(
            inputs[:, sl, :].transpose(1, 2, 0).reshape(I_LOC * J, B))
        # wt: [(i j), (n d)] ; wt[(i,j),(n,d)] = W[n, i, d, j]
        w_t = np.ascontiguousarray(
            W[:, sl, :, :].transpose(1, 3, 0, 2).reshape(I_LOC * J, ND))
        xh, xl = _hi_lo(x_t)
        wh, wl = _hi_lo(w_t)
        in_maps.append({"xth": np.ascontiguousarray(xh),
                        "xtl": np.ascontiguousarray(xl),
                        "wth": np.ascontiguousarray(wh),
                        "wtl": np.ascontiguousarray(wl)})
    return in_maps


def _ensure_ntff_hook():
    """Register the axon NTFF profile hook if the image's antenv lacks it."""
    import types

    try:
        import antenv.axon_hooks  # noqa: F401
        return
    except ImportError:
        pass
    import antenv

    if "/root/.axon_site" not in sys.path:
        sys.path.insert(0, "/root/.axon_site")
    from trn_agent_boot.trn_boot import _ntff_profile_via_ctypes

    hook = {"h": _ntff_profile_via_ctypes("/opt/axon/libaxon_pjrt.so")}
    mod = types.ModuleType("antenv.axon_hooks")
    mod.get_axon_ntff_profile_hook = lambda: hook["h"]
    mod.set_axon_ntff_profile_hook = lambda h: hook.__setitem__("h", h)
    sys.modules["antenv.axon_hooks"] = mod
    antenv.axon_hooks = mod


def run(inputs, W, trace=False):
    nc = _get_nc()
    if trace:
        _ensure_ntff_hook()
        # zero-egress container: skip the artifact upload, keep files local
        import concourse.bass_utils as bu
        bu.upload_artifacts = lambda d: d
    res = run_bass_kernel_spmd(
        nc, _make_in_maps(inputs, W), core_ids=list(range(CORES)),
        trace=trace,
    )
    return res.results[0]["out"].reshape(B, N, D), res


def kernel(inputs, W, routings=R, **_unused):
    assert int(routings) == R
    out, _ = run(inputs, W, trace=False)
    return out



# revision 4
# speedup vs baseline: 2.4955x; 2.4955x over previous
"""CapsuleLayer dynamic-routing kernel for Trainium2 (8 NeuronCores).

Problem: inputs [B=32, I=2048, J=16], W [N=64, I=2048, D=32, J=16], routings=3.
  inputs_hat[b,n,i,d] = sum_j inputs[b,i,j] * W[n,i,d,j]
  3 rounds of routing (softmax over n, weighted sum over i, squash over d).

Strategy: shard the input-capsule axis I across the 8 cores (I_loc=256).
All matmuls single-product bf16 (harness gate is rel_err < 2e-2; bf16 gives
~1e-3).  Free-dim order is (d, n) everywhere so that c-broadcast multiplies
keep innermost step=1 (DVE 2x bf16 mode).

Round 0 (c uniform): s0 = (1/N) sum_{ij} x W via K=128 fused matmuls,
4-way column-tiled into 4 replica strips, collapsed with a selector matmul.

Rounds 1-2, per group of 4 capsules i:
  PE 16-tile mains (row=free-quarter, col=capsule): H[(4i,32b),(32d,64n)] fp32
  ACT: evacuate H psum -> SBUF bf16 halves
  DVE: y = sum_d H*vb (bf16 2x mul + halving tree); b += y
  ACT/DVE (batched per 8 groups): c = softmax_n(b)
  DVE: tmp2 = c*H (bf16 2x, c broadcast on outer d axis)
  PE: srep[32,2048] += sel.T @ tmp2 (fold partitions and i)
Then AllReduce s (256 KB), squash on-chip, vb bf16 broadcast tile.
Host reassembles [B,D,N] -> [B,N,D].
"""

import sys

for p in ("/opt/trn_rl_repo",):
    if p not in sys.path:
        sys.path.insert(0, p)

import ml_dtypes
import numpy as np

import concourse.bacc as bacc
import concourse.mybir as mybir
import concourse.tile as tile
from concourse.bass_utils import run_bass_kernel_spmd

# problem constants (hardcoded per harness contract)
B, N, I, D, J = 32, 64, 2048, 32, 16
R = 3  # routings
CORES = 8
I_LOC = I // CORES  # 256
DN = D * N  # 2048
EPS = 1e-7

F32 = mybir.dt.float32
BF16 = mybir.dt.bfloat16
FX = mybir.AxisListType.X
ADD = mybir.AluOpType.add
ACT = mybir.ActivationFunctionType

G = I_LOC // 4  # 64 groups of 4 capsules per round
SBATCH = 8      # softmax batch (groups)
FBATCH = 4      # fold flush batch (groups)
HF = DN // 2    # 1024 free-dim half (d 0-15 / d 16-31)


def _squash_build(nc, vbp, sp, kp, s4, eps_ap, out32=None):
    """s4: [128, 2048] f32 (d,n)-order s replicated x4 on partition groups.
    Returns vb [128, 2048] bf16 = squash(s).  If out32 given ([32,2048] f32
    tile), also writes fp32 squash for rows 0-31 (the host output)."""
    sqf = kp.tile([128, DN], F32, tag="sq_sqf", bufs=1)
    nc.scalar.activation(sqf[:], s4[:], ACT.Square)
    # halving tree over outer d: flat halves coincide with d halves
    cur = sqf
    w = DN // 2
    while w >= N:
        nxt = kp.tile([128, w], F32, tag=f"sq_t{w}", bufs=1)
        nc.vector.tensor_add(nxt[:], cur[:, 0:w], cur[:, w:2 * w])
        cur = nxt
        w //= 2
    sq = cur  # [128, 64] f32 = sum_d s^2 per n
    t = sp.tile([128, N], F32, tag="sq_t", bufs=1)
    nc.scalar.activation(t[:], sq[:], ACT.Sqrt, bias=eps_ap)
    q1 = sp.tile([128, N], F32, tag="sq_q1", bufs=1)
    nc.scalar.activation(q1[:], sq[:], ACT.Identity, bias=1.0)
    den = sp.tile([128, N], F32, tag="sq_den", bufs=1)
    nc.vector.tensor_mul(den[:], q1[:], t[:])
    rs = sp.tile([128, N], F32, tag="sq_rs", bufs=1)
    nc.vector.reciprocal(rs[:], den[:])
    scale = sp.tile([128, N], F32, tag="sq_scale", bufs=1)
    nc.vector.tensor_mul(scale[:], sq[:], rs[:])
    vb = vbp.tile([128, DN], BF16, tag="sq_vb")
    nc.vector.tensor_mul(
        vb[:].rearrange("p (d n) -> p d n", n=N),
        s4[:].rearrange("p (d n) -> p d n", n=N),
        scale[:, None, :].broadcast_to([128, D, N]),
    )
    if out32 is not None:
        nc.vector.tensor_mul(
            out32[:].rearrange("p (d n) -> p d n", n=N),
            s4[0:32, :].rearrange("p (d n) -> p d n", n=N),
            scale[0:32, None, :].broadcast_to([32, D, N]),
        )
    return vb


def build_kernel():
    nc = bacc.Bacc("TRN2", target_bir_lowering=False, debug=False)

    xt = nc.dram_tensor("xt", [I_LOC * J, B], BF16, kind="ExternalInput")
    wt = nc.dram_tensor("wt", [I_LOC * J, DN], BF16, kind="ExternalInput")
    out = nc.dram_tensor("out", [B, DN], F32, kind="ExternalOutput")

    # collective bounce buffers (one pair per round)
    s_in = [nc.dram_tensor(f"s_in{r}", [B, DN], F32) for r in range(R)]
    s_out = [nc.dram_tensor(f"s_out{r}", [B, DN], F32, addr_space="Shared")
             for r in range(R)]

    with tile.TileContext(nc) as tc:
        with (
            tc.tile_pool(name="persist", bufs=1) as pp,
            tc.tile_pool(name="wr0", bufs=2) as wr0p,      # r0 W: [128,2,2048]b
            tc.tile_pool(name="wg", bufs=4) as wgp,        # rounds W: [128,4,512]b
            tc.tile_pool(name="hs", bufs=16) as hsp,       # evac'd H halves b
            tc.tile_pool(name="vbp", bufs=1) as vbp,
            tc.tile_pool(name="work", bufs=4) as kp,       # tree/work tiles
            tc.tile_pool(name="t2", bufs=12) as t2p,       # tmp2 halves
            tc.tile_pool(name="cst", bufs=2) as cstp,
            tc.tile_pool(name="small", bufs=4) as sp,
            tc.tile_pool(name="psH", bufs=2, space="PSUM") as psH,   # [128,1024]f32
            tc.tile_pool(name="psS", bufs=1, space="PSUM") as psS,   # [32,2048]f32
        ):
            # ---- resident tiles ----
            # x chunks for round-0 fused einsum: [128=(8i,16j), 32 chunks, B]
            xsb = pp.tile([128, I_LOC * J // 128, B], BF16, tag="xsb")
            nc.sync.dma_start(
                xsb[:], xt[:].rearrange("(k p) b -> p k b", p=128))
            # x for 16-tile mains: partitions 32q+j hold x[b, i, j], q=0..3
            xa4 = pp.tile([128, I_LOC, B], BF16, tag="xa4")
            for q in range(4):
                nc.sync.dma_start(
                    xa4[32 * q:32 * q + 16, :, :],
                    xt[:].rearrange("(i j) b -> j i b", j=J))

            # routing logits b: [128=(c,b), G, N] fp32
            bstate = pp.tile([128, G, N], F32, tag="bstate")
            nc.gpsimd.memset(bstate[:], 0.0)
            eps_t = pp.tile([128, 1], F32, tag="eps")
            nc.gpsimd.memset(eps_t[:], EPS)
            # selector[p, m] = 1.0 if p % 32 == m  (partition-group fold)
            sel_i = pp.tile([128, B], mybir.dt.int32, tag="sel_i")
            nc.gpsimd.iota(sel_i[:], [[1, B]], channel_multiplier=-1)
            nc.vector.tensor_scalar(sel_i[:], sel_i[:], 31, None,
                                    op0=mybir.AluOpType.bitwise_and)
            sel32 = pp.tile([128, B], F32, tag="sel32")
            nc.vector.tensor_scalar(sel32[:], sel_i[:], 0, None,
                                    op0=mybir.AluOpType.is_equal)
            selb = pp.tile([128, B], BF16, tag="selb")
            nc.vector.tensor_copy(selb[:], sel32[:])

            s4 = pp.tile([128, DN], F32, tag="s4")
            s_loc = pp.tile([32, DN], F32, tag="s_loc")
            out32 = pp.tile([32, DN], F32, tag="out32")

            wt_v = wt[:].rearrange("(i j) b -> j i b", j=J)  # noqa (unused)

            # ---------- round 0: c uniform -> s0 = (1/N) sum_i ihat ----------
            # psum accumulator: 4 column strips in two [128,1024] tiles
            pa = psH.tile([128, HF], F32, tag="ph")
            pb = psH.tile([128, HF], F32, tag="ph")
            n_chunks = I_LOC * J // 128  # 32
            for kb in range(n_chunks // 2):
                wsb = wr0p.tile([128, 2, DN], BF16, tag="wr0")
                nc.sync.dma_start(
                    wsb[:],
                    wt[256 * kb:256 * (kb + 1), :].rearrange(
                        "(c p) f -> p c f", p=128))
                for c2 in range(2):
                    k = 2 * kb + c2
                    strip = k % 4
                    for q in range(4):
                        ps = pa if q < 2 else pb
                        nc.tensor.matmul(
                            ps[32 * strip:32 * strip + 32,
                               512 * (q % 2):512 * (q % 2) + 512],
                            xsb[:, k, :],
                            wsb[:, c2, 512 * q:512 * q + 512],
                            start=(k < 4), stop=(k >= n_chunks - 4),
                            tile_position=(0, 32 * strip),
                            skip_group_check=True,
                        )
            # evacuate replica strips -> bf16, collapse via selector matmul
            s0b = pp.tile([128, DN], BF16, tag="s0b")
            nc.scalar.copy(s0b[:, 0:HF], pa[:])
            nc.scalar.copy(s0b[:, HF:DN], pb[:])
            s_ps = psS.tile([32, DN], F32, tag="sps")
            for q in range(4):
                nc.tensor.matmul(
                    s_ps[:, 512 * q:512 * q + 512],
                    selb[:], s0b[:, 512 * q:512 * q + 512],
                    start=True, stop=True,
                )
            nc.scalar.mul(s_loc[:], s_ps[:], 1.0 / N)
            nc.sync.dma_start(s_in[0][:], s_loc[:])
            nc.gpsimd.collective_compute(
                "AllReduce", ADD,
                replica_groups=[list(range(CORES))],
                ins=[s_in[0].ap().opt()], outs=[s_out[0].ap().opt()],
            )
            for g4 in range(4):
                nc.sync.dma_start(s4[32 * g4:32 * (g4 + 1), :], s_out[0][:])
            vb = _squash_build(nc, vbp, sp, kp, s4, eps_t[:])

            # ---------- rounds 1, 2 ----------
            for r in (1, 2):
                srep = psS.tile([32, DN], F32, tag="sps")
                pend_tmp2 = []  # (g, hs0, hs1) awaiting softmax c
                pend_fold = []  # (g, tm0, tm1) awaiting fold flush

                def flush_folds(last=False, _srep=srep):
                    while pend_fold:
                        g0, tms = pend_fold.pop(0)
                        for h in range(2):
                            for q in range(2):
                                f0 = HF * h + 512 * q
                                nc.tensor.matmul(
                                    _srep[:, f0:f0 + 512],
                                    selb[:],
                                    tms[h][:, 512 * q:512 * q + 512],
                                    start=(g0 == 0),
                                    stop=(last and not pend_fold
                                          and h == 1 and q == 1),
                                    skip_group_check=True,
                                )

                def do_softmax_batch(gb0, _r=r):
                    # c = softmax_n(b) for groups gb0..gb0+SBATCH-1
                    e = sp.tile([128, SBATCH, N], BF16, tag="e")
                    nc.scalar.activation(
                        e[:], bstate[:, gb0:gb0 + SBATCH, :], ACT.Exp)
                    se = sp.tile([128, SBATCH], F32, tag="se")
                    nc.vector.tensor_reduce(se[:], e[:], axis=FX, op=ADD)
                    rcp = sp.tile([128, SBATCH], F32, tag="rcp")
                    nc.vector.reciprocal(rcp[:], se[:])
                    cst = cstp.tile([128, SBATCH, N], BF16, tag="cst")
                    nc.vector.tensor_mul(
                        cst[:], e[:],
                        rcp[:, :, None].broadcast_to([128, SBATCH, N]))
                    # tmp2 = c * H for the groups that waited on this batch
                    while pend_tmp2:
                        gg, hs0, hs1 = pend_tmp2.pop(0)
                        gi = gg - gb0
                        tms = []
                        for h, hsx in enumerate((hs0, hs1)):
                            tm = t2p.tile([128, HF], BF16, tag="tm2")
                            nc.vector.tensor_mul(
                                tm[:].rearrange("p (d n) -> p d n", n=N),
                                hsx[:].rearrange("p (d n) -> p d n", n=N),
                                cst[:, gi, None, :].broadcast_to(
                                    [128, D // 2, N]),
                            )
                            tms.append(tm)
                        pend_fold.append((gg, tms))

                for g in range(G):
                    # W rows for capsules 4g..4g+3, strip-padded layout:
                    # partitions 32q+j hold quarter q (512 (d,n)-cols)
                    wg = wgp.tile([128, 4, 512], BF16, tag="wg")
                    for q in range(4):
                        nc.sync.dma_start(
                            wg[32 * q:32 * q + 16, :, :],
                            wt[64 * g:64 * g + 64,
                               512 * q:512 * q + 512].rearrange(
                                "(c j) f -> j c f", j=J))
                    ph0 = psH.tile([128, HF], F32, tag="ph")
                    ph1 = psH.tile([128, HF], F32, tag="ph")
                    phs = (ph0, ph1)
                    # 16-tile mains: row strip q = free-quarter, col strip c
                    for q in range(4):
                        for c in range(4):
                            nc.tensor.matmul(
                                phs[q // 2][32 * c:32 * c + 32,
                                            512 * (q % 2):512 * (q % 2) + 512],
                                xa4[32 * q:32 * q + 16, 4 * g + c, :],
                                wg[32 * q:32 * q + 16, c, :],
                                start=True, stop=True,
                                tile_position=(32 * q, 32 * c),
                            )
                    # evacuate H -> SBUF bf16 (ScalarE)
                    hs0 = hsp.tile([128, HF], BF16, tag="hs")
                    nc.scalar.copy(hs0[:], ph0[:])
                    hs1 = hsp.tile([128, HF], BF16, tag="hs")
                    nc.scalar.copy(hs1[:], ph1[:])
                    # y = sum_d H*vb : bf16 2x mul + halving tree
                    ty0 = kp.tile([128, HF], BF16, tag="ty0", bufs=2)
                    nc.vector.tensor_mul(ty0[:], hs0[:], vb[:, 0:HF])
                    ty1 = kp.tile([128, HF], BF16, tag="ty1", bufs=2)
                    nc.vector.tensor_mul(ty1[:], hs1[:], vb[:, HF:DN])
                    m8a = kp.tile([128, 512], BF16, tag="m8a", bufs=2)
                    nc.vector.tensor_add(m8a[:], ty0[:, 0:512], ty0[:, 512:HF])
                    m8b = kp.tile([128, 512], BF16, tag="m8b", bufs=2)
                    nc.vector.tensor_add(m8b[:], ty1[:, 0:512], ty1[:, 512:HF])
                    m4 = kp.tile([128, 512], BF16, tag="m4", bufs=2)
                    nc.vector.tensor_add(m4[:], m8a[:], m8b[:])
                    m2 = kp.tile([128, 256], BF16, tag="m2", bufs=2)
                    nc.vector.tensor_add(m2[:], m4[:, 0:256], m4[:, 256:512])
                    m1 = kp.tile([128, 128], BF16, tag="m1", bufs=2)
                    nc.vector.tensor_add(m1[:], m2[:, 0:128], m2[:, 128:256])
                    y = sp.tile([128, N], F32, tag="y")
                    nc.vector.tensor_add(y[:], m1[:, 0:N], m1[:, N:128])
                    # b += y
                    bsl = bstate[:, g, :]
                    nc.vector.tensor_add(bsl, bsl, y[:])
                    pend_tmp2.append((g, hs0, hs1))
                    if (g + 1) % SBATCH == 0:
                        do_softmax_batch(g + 1 - SBATCH)
                    if len(pend_fold) >= FBATCH:
                        flush_folds()
                flush_folds(last=True)

                # s partial -> DRAM -> AllReduce
                nc.scalar.copy(s_loc[:], srep[:])
                nc.sync.dma_start(s_in[r][:], s_loc[:])
                nc.gpsimd.collective_compute(
                    "AllReduce", ADD,
                    replica_groups=[list(range(CORES))],
                    ins=[s_in[r].ap().opt()], outs=[s_out[r].ap().opt()],
                )
                for g4 in range(4):
                    nc.sync.dma_start(s4[32 * g4:32 * (g4 + 1), :],
                                      s_out[r][:])
                vb = _squash_build(nc, vbp, sp, kp, s4, eps_t[:],
                                   out32=(out32 if r == 2 else None))

            nc.sync.dma_start(out[:], out32[:])

    nc.compile()
    return nc


_NC_CACHE = {}


def _get_nc():
    if "nc" not in _NC_CACHE:
        _NC_CACHE["nc"] = build_kernel()
    return _NC_CACHE["nc"]


def _make_in_maps(inputs, W):
    inputs = np.ascontiguousarray(np.asarray(inputs, dtype=np.float32))
    W = np.ascontiguousarray(np.asarray(W, dtype=np.float32))
    assert inputs.shape == (B, I, J) and W.shape == (N, I, D, J)
    in_maps = []
    for c in range(CORES):
        sl = slice(c * I_LOC, (c + 1) * I_LOC)
        # xt: [(i j), b]
        x_t = inputs[:, sl, :].transpose(1, 2, 0).reshape(I_LOC * J, B)
        # wt: [(i j), (d n)] ; wt[(i,j),(d,n)] = W[n, i, d, j]
        w_t = W[:, sl, :, :].transpose(1, 3, 2, 0).reshape(I_LOC * J, DN)
        in_maps.append({
            "xt": np.ascontiguousarray(x_t.astype(ml_dtypes.bfloat16)),
            "wt": np.ascontiguousarray(w_t.astype(ml_dtypes.bfloat16)),
        })
    return in_maps


def _ensure_ntff_hook():
    """Register the axon NTFF profile hook if the image's antenv lacks it."""
    import types

    try:
        import antenv.axon_hooks  # noqa: F401
        return
    except ImportError:
        pass
    import antenv

    if "/root/.axon_site" not in sys.path:
        sys.path.insert(0, "/root/.axon_site")
    from trn_agent_boot.trn_boot import _ntff_profile_via_ctypes

    hook = {"h": _ntff_profile_via_ctypes("/opt/axon/libaxon_pjrt.so")}
    mod = types.ModuleType("antenv.axon_hooks")
    mod.get_axon_ntff_profile_hook = lambda: hook["h"]
    mod.set_axon_ntff_profile_hook = lambda h: hook.__setitem__("h", h)
    sys.modules["antenv.axon_hooks"] = mod
    antenv.axon_hooks = mod


def run(inputs, W, trace=False):
    nc = _get_nc()
    if trace:
        _ensure_ntff_hook()
        # zero-egress container: skip the artifact upload, keep files local
        import concourse.bass_utils as bu
        bu.upload_artifacts = lambda d: d
    res = run_bass_kernel_spmd(
        nc, _make_in_maps(inputs, W), core_ids=list(range(CORES)),
        trace=trace,
    )
    o = res.results[0]["out"].reshape(B, D, N)
    return np.ascontiguousarray(o.transpose(0, 2, 1)), res


def kernel(inputs, W, routings=R, **_unused):
    assert int(routings) == R
    out, _ = run(inputs, W, trace=False)
    return out


# revision 6
# speedup vs baseline: 2.9970x; 1.2010x over previous
"""CapsuleLayer dynamic-routing kernel for Trainium2 (8 NeuronCores).

Problem: inputs [B=32, I=2048, J=16], W [N=64, I=2048, D=32, J=16], routings=3.
  inputs_hat[b,n,i,d] = sum_j inputs[b,i,j] * W[n,i,d,j]
  3 rounds of routing (softmax over n, weighted sum over i, squash over d).

Strategy: shard the input-capsule axis I across the 8 cores (I_loc=256).
All matmuls single-product bf16 (harness gate is rel_err < 2e-2).  Free-dim
order is (d, n) everywhere so c-broadcast multiplies keep innermost step=1
(DVE 2x bf16 mode).  Host pre-arranges every SBUF layout so all DMAs are
contiguous 4KB+ descriptors.

Round 0 (c uniform): s0 = (1/N) sum_{ij} x W via K=128 fused matmuls,
4-way column-tiled into 4 replica strips of srep, collapsed by a selector
matmul.  Rounds 1-2, per group of 4 capsules i:
  PE 16-tile mains (row=free-quarter, col=capsule): H[(4i,32b),(32d,64n)] f32
  ACT: evacuate H psum -> one SBUF bf16 tile per group
  DVE: y = sum_d H*vb (bf16 2x mul + 5-level halving tree); b += y
  (batched per 4 groups): c = softmax_n(b)
  DVE: tmp2 = c*H (bf16 2x, c broadcast on outer d axis)
  PE: srep[strip g%4] += sel.T @ tmp2 (fold partitions+i, 4-strip concurrent)
Then collapse strips, AllReduce s in bf16 (128 KB), squash on-chip.
Host reassembles [B,D,N] -> [B,N,D].
"""

import sys

for p in ("/opt/trn_rl_repo",):
    if p not in sys.path:
        sys.path.insert(0, p)

import ml_dtypes
import numpy as np

import concourse.bacc as bacc
import concourse.mybir as mybir
import concourse.tile as tile
from concourse.bass_utils import run_bass_kernel_spmd

# problem constants (hardcoded per harness contract)
B, N, I, D, J = 32, 64, 2048, 32, 16
R = 3  # routings
CORES = 8
I_LOC = I // CORES  # 256
DN = D * N  # 2048
EPS = 1e-7

F32 = mybir.dt.float32
BF16 = mybir.dt.bfloat16
FX = mybir.AxisListType.X
ADD = mybir.AluOpType.add
ACT = mybir.ActivationFunctionType

G = I_LOC // 4  # 64 groups of 4 capsules per round
SBATCH = 4      # softmax batch (groups)
FBATCH = 4      # fold flush batch (groups)
HF = DN // 2


def _squash_build(nc, vbp, sp, kp, s4, eps_ap, out32=None):
    """s4: [128, 2048] bf16 (d,n)-order s replicated x4 on partition groups.
    Returns vb [128, 2048] bf16 = squash(s).  If out32 given ([32,2048] f32
    tile), also writes fp32 squash for rows 0-31 (the host output)."""
    sqf = kp.tile([128, DN], F32, tag="sq_sqf", bufs=1)
    nc.scalar.activation(sqf[:], s4[:], ACT.Square)
    # halving tree over outer d: flat halves coincide with d halves
    cur = sqf
    w = DN // 2
    while w >= N:
        nxt = kp.tile([128, w], F32, tag=f"sq_t{w}", bufs=1)
        nc.vector.tensor_add(nxt[:], cur[:, 0:w], cur[:, w:2 * w])
        cur = nxt
        w //= 2
    sq = cur  # [128, 64] f32 = sum_d s^2 per n
    t = sp.tile([128, N], F32, tag="sq_t", bufs=1)
    nc.scalar.activation(t[:], sq[:], ACT.Sqrt, bias=eps_ap)
    q1 = sp.tile([128, N], F32, tag="sq_q1", bufs=1)
    nc.scalar.activation(q1[:], sq[:], ACT.Identity, bias=1.0)
    den = sp.tile([128, N], F32, tag="sq_den", bufs=1)
    nc.vector.tensor_mul(den[:], q1[:], t[:])
    rs = sp.tile([128, N], F32, tag="sq_rs", bufs=1)
    nc.vector.reciprocal(rs[:], den[:])
    scale = sp.tile([128, N], F32, tag="sq_scale", bufs=1)
    nc.vector.tensor_mul(scale[:], sq[:], rs[:])
    vb = vbp.tile([128, DN], BF16, tag="sq_vb")
    nc.vector.tensor_mul(
        vb[:].rearrange("p (d n) -> p d n", n=N),
        s4[:].rearrange("p (d n) -> p d n", n=N),
        scale[:, None, :].broadcast_to([128, D, N]),
    )
    if out32 is not None:
        nc.vector.tensor_mul(
            out32[:].rearrange("p (d n) -> p d n", n=N),
            s4[0:32, :].rearrange("p (d n) -> p d n", n=N),
            scale[0:32, None, :].broadcast_to([32, D, N]),
        )
    return vb


def build_kernel():
    nc = bacc.Bacc("TRN2", target_bir_lowering=False, debug=False)

    # host-prearranged inputs (all DMAs contiguous per partition):
    # xk[p, k, b] = x[b, i, j] with (i,j) = 128k+p      -- round-0 stationaries
    # xq[32q+j, i, b] = x[b, i, j], replicated q=0..3   -- 16-tile stationaries
    # wt[(i j), (d n)] = W[n, i, d, j]                  -- round-0 moving
    # wtp[32q+j, g, c, f] = wt[(4g+c) 16+j, 512q+f]     -- strip-padded moving
    xk = nc.dram_tensor("xk", [128, I_LOC * J // 128, B], BF16,
                        kind="ExternalInput")
    xq = nc.dram_tensor("xq", [128, I_LOC, B], BF16, kind="ExternalInput")
    wt = nc.dram_tensor("wt", [I_LOC * J, DN], BF16, kind="ExternalInput")
    wtp = nc.dram_tensor("wtp", [128, G, 4, 512], BF16, kind="ExternalInput")
    out = nc.dram_tensor("out", [B, DN], F32, kind="ExternalOutput")

    # collective bounce buffers (one pair per round), bf16 payload
    s_in = [nc.dram_tensor(f"s_in{r}", [B, DN], BF16) for r in range(R)]
    s_out = [nc.dram_tensor(f"s_out{r}", [B, DN], BF16, addr_space="Shared")
             for r in range(R)]

    with tile.TileContext(nc) as tc:
        with (
            tc.tile_pool(name="persist", bufs=1) as pp,
            tc.tile_pool(name="wr0", bufs=2) as wr0p,   # r0 W: [128,2,2048]b
            tc.tile_pool(name="wg", bufs=4) as wgp,     # rounds W: [128,4,512]b
            tc.tile_pool(name="hs", bufs=10) as hsp,    # evac'd H [128,2048]b
            tc.tile_pool(name="vbp", bufs=1) as vbp,
            tc.tile_pool(name="work", bufs=2) as kp,    # tree/work tiles
            tc.tile_pool(name="t2", bufs=6) as t2p,     # tmp2 [128,2048]b
            tc.tile_pool(name="cst", bufs=2) as cstp,
            tc.tile_pool(name="small", bufs=4) as sp,
            tc.tile_pool(name="psH", bufs=2, space="PSUM") as psH,  # [128,1024]
            tc.tile_pool(name="psS", bufs=1, space="PSUM") as psS,  # [128,2048]
        ):
            # ---- resident tiles (contiguous one-shot DMAs) ----
            xsb = pp.tile([128, I_LOC * J // 128, B], BF16, tag="xsb")
            nc.sync.dma_start(xsb[:], xk[:])
            xa4 = pp.tile([128, I_LOC, B], BF16, tag="xa4")
            nc.sync.dma_start(xa4[:], xq[:])

            bstate = pp.tile([128, G, N], F32, tag="bstate")
            nc.gpsimd.memset(bstate[:], 0.0)
            eps_t = pp.tile([128, 1], F32, tag="eps")
            nc.gpsimd.memset(eps_t[:], EPS)
            # selector[p, m] = 1.0 if p % 32 == m  (partition-group fold)
            sel_i = pp.tile([128, B], mybir.dt.int32, tag="sel_i")
            nc.gpsimd.iota(sel_i[:], [[1, B]], channel_multiplier=-1)
            nc.vector.tensor_scalar(sel_i[:], sel_i[:], 31, None,
                                    op0=mybir.AluOpType.bitwise_and)
            sel32 = pp.tile([128, B], F32, tag="sel32")
            nc.vector.tensor_scalar(sel32[:], sel_i[:], 0, None,
                                    op0=mybir.AluOpType.is_equal)
            selb = pp.tile([128, B], BF16, tag="selb")
            nc.vector.tensor_copy(selb[:], sel32[:])

            s4 = pp.tile([128, DN], BF16, tag="s4")
            srb = pp.tile([128, DN], BF16, tag="srb")
            s_locb = pp.tile([32, DN], BF16, tag="s_locb")
            out32 = pp.tile([32, DN], F32, tag="out32")

            def collapse_and_reduce(r, srep, scale):
                # evac 4-strip replica psum -> bf16, fold strips, AllReduce
                nc.scalar.copy(srb[:], srep[:])
                cp0 = psH.tile([128, HF], F32, tag="ph")
                cp1 = psH.tile([128, HF], F32, tag="ph")
                cps = (cp0, cp1)
                for q in range(4):
                    nc.tensor.matmul(
                        cps[q // 2][0:32, 512 * (q % 2):512 * (q % 2) + 512],
                        selb[:], srb[:, 512 * q:512 * q + 512],
                        start=True, stop=True,
                    )
                for h in range(2):
                    nc.scalar.mul(s_locb[:, HF * h:HF * (h + 1)],
                                  cps[h][0:32, :], scale)
                nc.sync.dma_start(s_in[r][:], s_locb[:])
                nc.gpsimd.collective_compute(
                    "AllReduce", ADD,
                    replica_groups=[list(range(CORES))],
                    ins=[s_in[r].ap().opt()], outs=[s_out[r].ap().opt()],
                )
                for g4 in range(4):
                    nc.sync.dma_start(s4[32 * g4:32 * (g4 + 1), :],
                                      s_out[r][:])

            # ---------- round 0: c uniform -> s0 = (1/N) sum_i ihat ----------
            srep = psS.tile([128, DN], F32, tag="srep")
            n_chunks = I_LOC * J // 128  # 32
            for kb in range(n_chunks // 2):
                wsb = wr0p.tile([128, 2, DN], BF16, tag="wr0")
                nc.sync.dma_start(
                    wsb[:],
                    wt[256 * kb:256 * (kb + 1), :].rearrange(
                        "(c p) f -> p c f", p=128))
                for c2 in range(2):
                    k = 2 * kb + c2
                    strip = k % 4
                    for q in range(4):
                        nc.tensor.matmul(
                            srep[32 * strip:32 * strip + 32,
                                 512 * q:512 * q + 512],
                            xsb[:, k, :],
                            wsb[:, c2, 512 * q:512 * q + 512],
                            start=(k < 4), stop=(k >= n_chunks - 4),
                            tile_position=(0, 32 * strip),
                            skip_group_check=True,
                        )
            collapse_and_reduce(0, srep, 1.0 / N)
            vb = _squash_build(nc, vbp, sp, kp, s4, eps_t[:])

            # ---------- rounds 1, 2 ----------
            for r in (1, 2):
                srep = psS.tile([128, DN], F32, tag="srep")
                pend_tmp2 = []  # (g, hs) awaiting softmax c
                pend_fold = []  # (g, tm) awaiting fold flush

                def flush_folds(last=False, _srep=srep):
                    while pend_fold:
                        g0, tm = pend_fold.pop(0)
                        s0 = 32 * (g0 % 4)
                        for f in range(4):
                            nc.tensor.matmul(
                                _srep[s0:s0 + 32, 512 * f:512 * f + 512],
                                selb[:],
                                tm[:, 512 * f:512 * f + 512],
                                start=(g0 < 4),
                                stop=(g0 >= G - 4),
                                tile_position=(0, s0),
                                skip_group_check=True,
                            )

                def do_softmax_batch(gb0):
                    # c = softmax_n(b) for groups gb0..gb0+SBATCH-1
                    e = sp.tile([128, SBATCH, N], BF16, tag="e")
                    nc.scalar.activation(
                        e[:], bstate[:, gb0:gb0 + SBATCH, :], ACT.Exp)
                    se = sp.tile([128, SBATCH], F32, tag="se")
                    nc.vector.tensor_reduce(se[:], e[:], axis=FX, op=ADD)
                    rcp = sp.tile([128, SBATCH], F32, tag="rcp")
                    nc.vector.reciprocal(rcp[:], se[:])
                    cst = cstp.tile([128, SBATCH, N], BF16, tag="cst")
                    nc.vector.tensor_mul(
                        cst[:], e[:],
                        rcp[:, :, None].broadcast_to([128, SBATCH, N]))
                    # tmp2 = c * H for the groups that waited on this batch
                    while pend_tmp2:
                        gg, hsx = pend_tmp2.pop(0)
                        gi = gg - gb0
                        tm = t2p.tile([128, DN], BF16, tag="tm2")
                        nc.vector.tensor_mul(
                            tm[:].rearrange("p (d n) -> p d n", n=N),
                            hsx[:].rearrange("p (d n) -> p d n", n=N),
                            cst[:, gi, None, :].broadcast_to([128, D, N]),
                        )
                        pend_fold.append((gg, tm))

                for g in range(G):
                    # W rows for capsules 4g..4g+3, strip-padded layout
                    wg = wgp.tile([128, 4, 512], BF16, tag="wg")
                    nc.sync.dma_start(wg[:], wtp[:, g, :, :])
                    ph0 = psH.tile([128, HF], F32, tag="ph")
                    ph1 = psH.tile([128, HF], F32, tag="ph")
                    phs = (ph0, ph1)
                    # 16-tile mains: row strip q = free-quarter, col strip c
                    for q in range(4):
                        for c in range(4):
                            nc.tensor.matmul(
                                phs[q // 2][32 * c:32 * c + 32,
                                            512 * (q % 2):512 * (q % 2) + 512],
                                xa4[32 * q:32 * q + 16, 4 * g + c, :],
                                wg[32 * q:32 * q + 16, c, :],
                                start=True, stop=True,
                                tile_position=(32 * q, 32 * c),
                            )
                    # evacuate H -> SBUF bf16 (ScalarE), one tile per group
                    hs = hsp.tile([128, DN], BF16, tag="hs")
                    nc.scalar.copy(hs[:, 0:HF], ph0[:])
                    nc.scalar.copy(hs[:, HF:DN], ph1[:])
                    # y = sum_d H*vb : bf16 2x mul + 5-level halving tree
                    ty = kp.tile([128, DN], BF16, tag="ty")
                    nc.vector.tensor_mul(ty[:], hs[:], vb[:])
                    m16 = kp.tile([128, 1024], BF16, tag="m16")
                    nc.vector.tensor_add(m16[:], ty[:, 0:1024], ty[:, 1024:DN])
                    m8 = kp.tile([128, 512], BF16, tag="m8")
                    nc.vector.tensor_add(m8[:], m16[:, 0:512], m16[:, 512:1024])
                    m4 = kp.tile([128, 256], BF16, tag="m4")
                    nc.vector.tensor_add(m4[:], m8[:, 0:256], m8[:, 256:512])
                    m2 = kp.tile([128, 128], BF16, tag="m2")
                    nc.vector.tensor_add(m2[:], m4[:, 0:128], m4[:, 128:256])
                    y = sp.tile([128, N], F32, tag="y")
                    nc.vector.tensor_add(y[:], m2[:, 0:N], m2[:, N:128])
                    # b += y
                    bsl = bstate[:, g, :]
                    nc.vector.tensor_add(bsl, bsl, y[:])
                    pend_tmp2.append((g, hs))
                    if (g + 1) % SBATCH == 0:
                        do_softmax_batch(g + 1 - SBATCH)
                    if len(pend_fold) >= FBATCH:
                        flush_folds()
                flush_folds(last=True)
                collapse_and_reduce(r, srep, 1.0)
                vb = _squash_build(nc, vbp, sp, kp, s4, eps_t[:],
                                   out32=(out32 if r == 2 else None))

            nc.sync.dma_start(out[:], out32[:])

    nc.compile()
    return nc


_NC_CACHE = {}


def _get_nc():
    if "nc" not in _NC_CACHE:
        _NC_CACHE["nc"] = build_kernel()
    return _NC_CACHE["nc"]


def _make_in_maps(inputs, W):
    inputs = np.ascontiguousarray(np.asarray(inputs, dtype=np.float32))
    W = np.ascontiguousarray(np.asarray(W, dtype=np.float32))
    assert inputs.shape == (B, I, J) and W.shape == (N, I, D, J)
    in_maps = []
    for c in range(CORES):
        sl = slice(c * I_LOC, (c + 1) * I_LOC)
        x_t = inputs[:, sl, :].transpose(1, 2, 0)  # [i, j, b]
        # xk[p, k, b]: (i,j) = 128k+p
        x_k = x_t.reshape(I_LOC * J, B).reshape(32, 128, B).transpose(1, 0, 2)
        # xq[32q+j, i, b], q-replicated with 16-row padding
        x_jib = x_t.transpose(1, 0, 2)  # [j, i, b]
        x_q = np.zeros((4, 32, I_LOC, B), dtype=np.float32)
        x_q[:, 0:16] = x_jib[None, :, :, :]
        x_q = x_q.reshape(128, I_LOC, B)
        # wt[(i j), (d n)] = W[n, i, d, j]
        w_t = W[:, sl, :, :].transpose(1, 3, 2, 0).reshape(I_LOC * J, DN)
        # wtp[32q+j, g, c, f] = wt[(4g+c)16+j, 512q+f]
        w_4 = w_t.reshape(G, 4, J, 4, 512)  # [g, c, j, q, f]
        w_p = np.zeros((4, 32, G, 4, 512), dtype=np.float32)
        w_p[:, 0:16] = w_4.transpose(3, 2, 0, 1, 4)
        w_p = w_p.reshape(128, G, 4, 512)
        bf = ml_dtypes.bfloat16
        in_maps.append({
            "xk": np.ascontiguousarray(x_k.astype(bf)),
            "xq": np.ascontiguousarray(x_q.astype(bf)),
            "wt": np.ascontiguousarray(w_t.astype(bf)),
            "wtp": np.ascontiguousarray(w_p.astype(bf)),
        })
    return in_maps


def _ensure_ntff_hook():
    """Register the axon NTFF profile hook if the image's antenv lacks it."""
    import types

    try:
        import antenv.axon_hooks  # noqa: F401
        return
    except ImportError:
        pass
    import antenv

    if "/root/.axon_site" not in sys.path:
        sys.path.insert(0, "/root/.axon_site")
    from trn_agent_boot.trn_boot import _ntff_profile_via_ctypes

    hook = {"h": _ntff_profile_via_ctypes("/opt/axon/libaxon_pjrt.so")}
    mod = types.ModuleType("antenv.axon_hooks")
    mod.get_axon_ntff_profile_hook = lambda: hook["h"]
    mod.set_axon_ntff_profile_hook = lambda h: hook.__setitem__("h", h)
    sys.modules["antenv.axon_hooks"] = mod
    antenv.axon_hooks = mod


def run(inputs, W, trace=False):
    nc = _get_nc()
    if trace:
        _ensure_ntff_hook()
        # zero-egress container: skip the artifact upload, keep files local
        import concourse.bass_utils as bu
        bu.upload_artifacts = lambda d: d
    res = run_bass_kernel_spmd(
        nc, _make_in_maps(inputs, W), core_ids=list(range(CORES)),
        trace=trace,
    )
    o = res.results[0]["out"].reshape(B, D, N)
    return np.ascontiguousarray(o.transpose(0, 2, 1)), res


def kernel(inputs, W, routings=R, **_unused):
    assert int(routings) == R
    out, _ = run(inputs, W, trace=False)
    return out
